# revision 3
# baseline (speedup 1.0000x reference)
"""Bass/Trainium2 kernel for nn_BmmEnsemble (ensemble-of-MLPs atomic energy sum).

Sharding: 8 cores; core c owns species c//2, half c%2 (12500/2 = 6250 atoms).
Per (ensemble member e, 256-atom block) the core runs the 3-layer MLP
(1008->256->192->160, CELU alpha=0.1) and reduces layer-3 activations to
per-feature atom sums; layer 4 / ensemble mean / final sum are linear and
done on the host in fp64.

Precision/engine plan (validated numerically, rel err ~7e-3):
  L1 on PE in fp8e4 DoubleRow mode (K=256/instr, 0.5 cyc/row): x single
    fp8, W1 split hi+lo fp8 (kills systematic weight-rounding error),
    bias+alpha rides a ones-row of x (split hi/lo like W1).
  L2/L3 on PE in bf16 (activations g are bf16: fp8 activations fail the
    2e-2 gate via celu-curvature rectification of rounding noise).
    L2 bias seeded into PSUM by a K=2 matmul of bf16 hi/lo bias rows
    against a ones tile; pad partitions seeded to 0.1 so g2's pad rows
    are the constant 0.1, which L3 uses as its bias ones-row (w3 rows
    64/65 of k-chunk1 = 10*b3_adj hi/lo).
  CELU: z' = z + b + alpha in PSUM; u = ACT Exp(10 z' + ln a - 1) -> bf16;
    g = DVE stt max(z', min(u, alpha)) -> bf16  (= celu(z+b) + alpha).
  L3 sums: stt with accum_out. PSUM layout packs L3 M-blocks so no lanes
    are wasted: C banks hold features 0:128 for an (even,odd) e pair;
    E banks hold features 128:160 as 4 partition stripes x 2 column
    halves = all 8 members of one block. GPSIMD/Pool is unusable (no
    PSUM access, no ALU ops on core v3).

Emission is software-pipelined: step t runs [L1(t), L2(t-1), L3(t-2)].
"""

import numpy as np
import ml_dtypes

import concourse.bacc as bacc
import concourse.tile as tile
import concourse.mybir as mybir
from concourse.bass_utils import run_bass_kernel_spmd

F32 = mybir.dt.float32
BF16 = mybir.dt.bfloat16
FP8 = mybir.dt.float8e4
AF = mybir.ActivationFunctionType
ALU = mybir.AluOpType
DR = mybir.MatmulPerfMode.DoubleRow

NPF8 = ml_dtypes.float8_e4m3
NPBF = ml_dtypes.bfloat16

S = 4
E = 8
N = 50000
AEV = 1008
ALPHA = 0.1
LNA1 = float(np.log(ALPHA) - 1.0)
NCORES = 8
NA = N // S // 2            # atoms per core: 6250
BLK = 256                   # atom block (DR moving limit: 2*256 = 512)
NB = (NA + BLK - 1) // BLK  # 25 blocks (24 x 256 + 106)
PADC = 0.1                  # constant seeded into L2 pad partitions -> g2 pad
NCOL_C = NB * E             # one accum col per (block, e): 200
NCOL = NCOL_C + NB * 2      # + (block, col-half) E cols: 250
NSLOT = NB * E              # 200 pipeline slots


def _build():
    nc = bacc.Bacc("TRN2", target_bir_lowering=False, debug=False,
                   num_devices=NCORES)

    x8 = nc.dram_tensor("x8", [4, 128, 2, NA], FP8, kind="ExternalInput")
    w1 = nc.dram_tensor("w1", [E, 2, 4, 128, 2, 256], FP8, kind="ExternalInput")
    w2 = nc.dram_tensor("w2", [E, 2, 128, 256], BF16, kind="ExternalInput")
    b2r = nc.dram_tensor("b2r", [E, 2, 256], BF16, kind="ExternalInput")
    w3 = nc.dram_tensor("w3", [E, 2, 128, 160], BF16, kind="ExternalInput")
    acc = nc.dram_tensor("acc", [128, NCOL], F32, kind="ExternalOutput")

    with tile.TileContext(nc) as tc:
        with (
            tc.tile_pool(name="wp", bufs=1) as wp,
            tc.tile_pool(name="xp", bufs=2) as xp,
            tc.tile_pool(name="up", bufs=2) as up,
            tc.tile_pool(name="gp", bufs=2) as gp,
            tc.tile_pool(name="ps", bufs=2, space="PSUM") as ps,
        ):
            # ---- resident weights ----
            w1t, w2t, b2t, w3t = {}, {}, {}, {}
            for e in range(E):
                for hl in range(2):
                    for kp in range(4):
                        t = wp.tile([128, 2, 256], FP8, tag=f"w1_{e}_{hl}_{kp}")
                        nc.sync.dma_start(t[:], w1[e, hl, kp])
                        w1t[e, hl, kp] = t
                for kc in range(2):
                    t = wp.tile([128, 256], BF16, tag=f"w2_{e}_{kc}")
                    nc.sync.dma_start(t[:], w2[e, kc])
                    w2t[e, kc] = t
                    t = wp.tile([128, 160], BF16, tag=f"w3_{e}_{kc}")
                    nc.sync.dma_start(t[:], w3[e, kc])
                    w3t[e, kc] = t
                t = wp.tile([2, 256], BF16, tag=f"b2_{e}")
                nc.sync.dma_start(t[:], b2r[e])
                b2t[e] = t
            ones = wp.tile([2, 512], BF16, tag="ones")
            nc.vector.memset(ones[:], 1.0)
            bexp = wp.tile([128, 1], F32, tag="bexp")
            nc.vector.memset(bexp[:], LNA1)
            acct = wp.tile([128, NCOL], F32, tag="acct")

            # ---- x prefetch ----
            xtiles = {}

            def emit_x_dma(b):
                if b >= NB or b in xtiles:
                    return
                na = min(BLK, NA - b * BLK)
                lst = []
                for kp in range(4):
                    t = xp.tile([128, 2, BLK], FP8, tag=f"x{kp}")
                    nc.sync.dma_start(t[:, :, :na],
                                      x8[kp, :, :, b * BLK:b * BLK + na])
                    lst.append(t)
                xtiles[b] = lst

            emit_x_dma(0)
            emit_x_dma(1)

            state = {}

            def slot(t):
                b, e = divmod(t, E)
                return b, e, min(BLK, NA - b * BLK)

            def l1mm(t):
                b, e, na = slot(t)
                if e == 0:
                    emit_x_dma(b + 1)
                st = state.setdefault(t, {})
                A = ps.tile([128, 512], F32, tag="A")
                st["A"] = A
                xt = xtiles[b]
                for m in range(2):
                    out = A[:, m * 256:m * 256 + na]
                    gi = 0
                    for hl in range(2):
                        for kp in range(4):
                            nc.tensor.matmul(
                                out,
                                w1t[e, hl, kp][:, :, m * 128:(m + 1) * 128],
                                xt[kp][:, :, :na],
                                start=(gi == 0), stop=(gi == 7),
                                perf_mode=DR)
                            gi += 1

            def l1ew(t):
                b, e, na = slot(t)
                st = state[t]
                A = st["A"]
                u = up.tile([128, 512], BF16, tag="u1")
                nc.scalar.activation(u[:], A[:], AF.Exp,
                                     bias=bexp[:, 0:1], scale=10.0)
                g = gp.tile([128, 512], BF16, tag="g1")
                nc.vector.scalar_tensor_tensor(g[:], u[:], ALPHA, A[:],
                                               op0=ALU.min, op1=ALU.max)
                st["g1"] = g

            def l2mm(t):
                b, e, na = slot(t)
                st = state[t]
                g1 = st["g1"]
                B = ps.tile([128, 512], F32, tag="B")
                st["B"] = B
                for m in range(2):
                    out = B[:, m * 256:m * 256 + na]
                    nc.tensor.matmul(out, b2t[e][:, m * 128:(m + 1) * 128],
                                     ones[:, :na], start=True, stop=False)
                    for kc in range(2):
                        nc.tensor.matmul(
                            out, w2t[e, kc][:, m * 128:(m + 1) * 128],
                            g1[:, kc * 256:kc * 256 + na],
                            start=False, stop=(kc == 1))

            def l2ew(t):
                b, e, na = slot(t)
                st = state[t]
                B = st["B"]
                u = up.tile([128, 512], BF16, tag="u2")
                nc.scalar.activation(u[:], B[:], AF.Exp,
                                     bias=bexp[:, 0:1], scale=10.0)
                g = gp.tile([128, 512], BF16, tag="g2")
                nc.vector.scalar_tensor_tensor(g[:], u[:], ALPHA, B[:],
                                               op0=ALU.min, op1=ALU.max)
                st["g2"] = g

            ctx = {}

            def l3mm(t):
                b, e, na = slot(t)
                st = state[t]
                g2 = st["g2"]
                if e % 2 == 0:
                    ctx["C"] = ps.tile([128, 512], F32, tag="C", name=f"C_{t}")
                if e == 0:
                    ctx["E"] = ps.tile([128, 512], F32, tag="Eb", name=f"E_{t}")
                C, Eb = ctx["C"], ctx["E"]
                st["C"], st["E"] = C, Eb
                co = (e % 2) * 256
                for kc in range(2):
                    nc.tensor.matmul(C[:, co:co + na],
                                     w3t[e, kc][:, 0:128],
                                     g2[:, kc * 256:kc * 256 + na],
                                     start=(kc == 0), stop=(kc == 1))
                sp = 32 * (e % 4)
                eo = (e // 4) * 256
                for kc in range(2):
                    nc.tensor.matmul(Eb[sp:sp + 32, eo:eo + na],
                                     w3t[e, kc][:, 128:160],
                                     g2[:, kc * 256:kc * 256 + na],
                                     start=(kc == 0), stop=(kc == 1),
                                     tile_position=(0, sp))

            def l3ew(t):
                b, e, na = slot(t)
                st = state[t]
                C, Eb = st["C"], st["E"]
                if e % 2 == 1:
                    u = up.tile([128, 512], BF16, tag="uC")
                    nc.scalar.activation(u[:], C[:], AF.Exp,
                                         bias=bexp[:, 0:1], scale=10.0)
                    s3 = gp.tile([128, 512], BF16, tag="s3")
                    for h in range(2):
                        eh = e - 1 + h
                        col = b * E + eh
                        nc.vector.scalar_tensor_tensor(
                            s3[:, h * 256:h * 256 + na],
                            u[:, h * 256:h * 256 + na], ALPHA,
                            C[:, h * 256:h * 256 + na],
                            op0=ALU.min, op1=ALU.max,
                            accum_out=acct[:, col:col + 1])
                if e == 7:
                    u = up.tile([128, 512], BF16, tag="uE")
                    nc.scalar.activation(u[:], Eb[:], AF.Exp,
                                         bias=bexp[:, 0:1], scale=10.0)
                    sE = gp.tile([128, 512], BF16, tag="sE")
                    for h in range(2):
                        col = NCOL_C + b * 2 + h
                        nc.vector.scalar_tensor_tensor(
                            sE[:, h * 256:h * 256 + na],
                            u[:, h * 256:h * 256 + na], ALPHA,
                            Eb[:, h * 256:h * 256 + na],
                            op0=ALU.min, op1=ALU.max,
                            accum_out=acct[:, col:col + 1])
                # free slot state
                del state[t]

            for t in range(NSLOT + 2):
                if t < NSLOT:
                    l1mm(t)
                    l1ew(t)
                if 1 <= t <= NSLOT:
                    l2mm(t - 1)
                    l2ew(t - 1)
                if t >= 2:
                    l3mm(t - 2)
                    l3ew(t - 2)

            nc.sync.dma_start(acc[:], acct[:])
    nc.compile()
    return nc


_NC = None


def _get_nc():
    global _NC
    if _NC is None:
        _NC = _build()
    return _NC


def _prep_inputs(inputs):
    aev = np.asarray(inputs["aev"], dtype=np.float32).reshape(N, AEV)
    idx = np.asarray(inputs["idx"])
    Ws = [np.asarray(inputs[f"W{i}"], dtype=np.float64) for i in (1, 2, 3, 4)]
    bs = [np.asarray(inputs[f"b{i}"], dtype=np.float64) for i in (1, 2, 3, 4)]

    in_maps = []
    for c in range(NCORES):
        s, h = c // 2, c % 2
        sel = np.asarray(idx[s, h * NA:(h + 1) * NA])
        xfull = np.zeros((1024, NA), dtype=np.float32)
        xfull[:AEV] = aev[sel].T
        xfull[AEV] = 1.0
        x8 = np.ascontiguousarray(
            xfull.reshape(4, 2, 128, NA).transpose(0, 2, 1, 3)).astype(NPF8)

        w1c = np.zeros((E, 2, 4, 128, 2, 256), dtype=NPF8)
        w2c = np.zeros((E, 2, 128, 256), dtype=NPBF)
        b2c = np.zeros((E, 2, 256), dtype=NPBF)
        w3c = np.zeros((E, 2, 128, 160), dtype=NPBF)
        for e in range(E):
            w1full = np.zeros((1024, 256), dtype=np.float64)
            w1full[:AEV] = Ws[0][s, e]
            w1full[AEV] = bs[0][s, e, 0] + ALPHA
            hi = w1full.astype(NPF8)
            lo = (w1full - hi.astype(np.float64)).astype(NPF8)
            for hl, w in enumerate((hi, lo)):
                w1c[e, hl] = w.astype(NPF8).reshape(4, 2, 128, 256) \
                    .transpose(0, 2, 1, 3)

            w2full = np.zeros((256, 256), dtype=np.float64)
            w2full[:, :192] = Ws[1][s, e]
            w2c[e] = w2full.astype(NPBF).reshape(2, 128, 256)

            b2a = np.full(256, PADC, dtype=np.float64)
            b2a[:192] = bs[1][s, e, 0] - ALPHA * Ws[1][s, e].sum(axis=0) + ALPHA
            bhi = b2a.astype(NPBF)
            blo = (b2a - bhi.astype(np.float64)).astype(NPBF)
            b2c[e, 0], b2c[e, 1] = bhi, blo

            w3full = np.zeros((2, 128, 160), dtype=np.float64)
            w3full[0] = Ws[2][s, e][0:128]
            w3full[1, 0:64] = Ws[2][s, e][128:192]
            b3a = bs[2][s, e, 0] - ALPHA * Ws[2][s, e].sum(axis=0) + ALPHA
            r64 = (b3a / PADC).astype(NPBF)
            r65 = ((b3a - PADC * r64.astype(np.float64)) / PADC).astype(NPBF)
            w3full[1, 64] = r64.astype(np.float64)
            w3full[1, 65] = r65.astype(np.float64)
            w3c[e] = w3full.astype(NPBF)

        in_maps.append({"x8": x8, "w1": w1c, "w2": w2c, "b2r": b2c,
                        "w3": w3c})
    return in_maps, Ws, bs


def _finish(results, Ws, bs):
    W4, b4 = Ws[3], bs[3]  # [S,E,160,1], [S,E,1,1]
    total = 0.0
    for c in range(NCORES):
        s = c // 2
        a = results[c]["acc"].astype(np.float64)  # [128, NCOL]
        for e in range(E):
            g3 = np.zeros(160)
            cols = [b * E + e for b in range(NB)]
            g3[0:128] = a[:, cols].sum(axis=1)
            hh = e // 4
            r0 = 32 * (e % 4)
            ecols = [NCOL_C + b * 2 + hh for b in range(NB)]
            g3[128:160] = a[r0:r0 + 32, ecols].sum(axis=1)
            h3 = g3 - ALPHA * NA
            total += (h3 @ W4[s, e, :, 0] + NA * b4[s, e, 0, 0]) / E
    return np.array([total], dtype=np.float32)


def _run(inputs, **spmd_kwargs):
    in_maps, Ws, bs = _prep_inputs(inputs)
    nc = _get_nc()
    res = run_bass_kernel_spmd(nc, in_maps, list(range(NCORES)), **spmd_kwargs)
    return _finish(res.results, Ws, bs), res


def kernel(**inputs) -> np.ndarray:
    out, _ = _run(inputs)
    return out
